# revision 2
# baseline (speedup 1.0000x reference)
"""Trainium2 Bass kernel for nn_Discriminator (2-layer GRU, H=512, B=256, T=2048).

Strategy: data-parallel over batch across 8 cores (32 rows each). Per core the
two GRU layers run as a sequential scan over T. Matmuls keep h as the
stationary operand (hT chunks [128,32]) and stream W^T as the moving operand,
with 4-way PE column tiling: col-group j computes the gates for h-columns
[128j, 128j+128) of every gate type, written to PSUM partitions [32j, 32j+32)
as blocks [r | z | hn | in] x 128 cols. All elementwise work then runs on full
128-partition tiles with free-dim offsets only. A single PE transpose per layer
returns n_pre (and z) to the transposed layout so the next step's stationary
operand needs no extra reshuffle.

Layouts per core:
  strip "S" [128, 128]: partition 32j+b, free f  <->  (batch b, h-col 128j+f)
  transp "T" [128, 128]: partition p, col 32k+b  <->  (h-col 128k+p, batch b)
"""

import json
import os
import ml_dtypes
import numpy as np

import concourse.bass as bass
import concourse.mybir as mybir
from concourse.tile import TileContext, ScopedClock
from concourse.bass_utils import run_bass_kernel_spmd


# --- BIR rewrite: this walrus build allows only 1 sync wait per instruction.
# Split each instruction's extra waits into preceding single-wait NOPs on the
# same engine (engine streams execute in block order, so semantics are
# preserved: all waits still complete before the instruction issues).
_MAX_WAITS = 1


def _split_sync_waits_json(bir_bytes):
    m = json.loads(bir_bytes)
    n_split = [0]

    def fix_block(block):
        insts = block.get("instructions")
        if insts:
            out = []
            for inst in insts:
                si = inst.get("sync_info")
                waits = (si or {}).get("on_wait") or []
                maxw = 0 if inst.get("opcode") == "Drain" else _MAX_WAITS
                if len(waits) > maxw:
                    keep = waits[-maxw:] if maxw else []
                    move = waits[:-maxw] if maxw else waits
                    for i, w in enumerate(move):
                        out.append({
                            "debug": inst.get("debug", 0),
                            "engine": inst["engine"],
                            "ins": [],
                            "name": f"{inst['name']}-ws{i}",
                            "opcode": "NoOp",
                            "outs": [],
                            "sync_info": {"on_update": [], "on_wait": [w]},
                        })
                    si["on_wait"] = keep
                    n_split[0] += 1
                out.append(inst)
            block["instructions"] = out
        for sub in block.get("blocks", []):
            fix_block(sub)

    for f in m["functions"]:
        for b in f["blocks"]:
            fix_block(b)
    return json.dumps(m).encode()


def _install_wait_split_patch():
    import concourse.bass_utils as bu
    import concourse.bass2jax as b2j
    if getattr(bu, "_gru_wait_split", False):
        return
    orig = bu.compile_bir_kernel

    def patched(bir_json, tmpdir, neff_name="file.neff"):
        return orig(_split_sync_waits_json(bir_json), tmpdir, neff_name)

    bu.compile_bir_kernel = patched
    bu._gru_wait_split = True
    if getattr(b2j, "compile_bir_kernel", None) is orig:
        b2j.compile_bir_kernel = patched


_install_wait_split_patch()

H = 512
BC = 32          # batch rows per core
N_CORES = 8
FP32 = mybir.dt.float32
BF16 = mybir.dt.bfloat16
AF = mybir.ActivationFunctionType
ALU = mybir.AluOpType
# matmul-operand dtype: bf16 streams 1 col/cycle on the PE (fp32 is 4x
# slower) and supports column tiling (f32r does not). End-to-end GRU error
# with bf16 operands + fp32 PSUM accumulate measures ~6e-4.
DT_MM = BF16
NP_MM = ml_dtypes.bfloat16


class PatchedTileContext(TileContext):
    """This walrus build rejects >1 sync wait on one TPB_CTRL instruction;
    split the tail drain's waits into single-wait NOPs."""

    def _drain_and_barrier(self, tick_clock, wait_clock):
        drain_inst = self.nc.sync.drain()
        wait_clock.add_sem_waits(
            drain_inst.ins, ScopedClock({None: tick_clock.global_clock})
        )
        si = drain_inst.ins.sync_info
        waits = list(si.on_wait) if si is not None else []
        if len(waits) > 1:
            si.on_wait = []
            for w in waits:
                nop = self.nc.sync.nop(nofuse=True, hint="drain_wait_split")
                nop.ins.sync_info = mybir.SyncInfo(on_wait=[w], on_update=[])

        self.nc.all_engine_barrier()
        assert self.sems is not None
        popped = self.nc._tile_sem_poison_stack.pop()
        assert popped is self._sem_poison
        self.nc.clear_and_free_semaphores(list(self.sems.allocated().values()))
        self.nc.all_engine_barrier()


def build_nc(T, U, repeat=1):
    nc = bass.Bass()

    xt = nc.dram_tensor("xt", [T, BC], DT_MM, kind="ExternalInput")
    w1s = nc.dram_tensor("w1s", [128, 4 * 3 * H], DT_MM, kind="ExternalInput")
    w2i = nc.dram_tensor("w2i", [128, 4 * 3 * H], DT_MM, kind="ExternalInput")
    w2h = nc.dram_tensor("w2h", [128, 4 * 3 * H], DT_MM, kind="ExternalInput")
    f1 = nc.dram_tensor("f1", [8, H], DT_MM, kind="ExternalInput")
    f2 = nc.dram_tensor("f2", [4, H], DT_MM, kind="ExternalInput")
    ident = nc.dram_tensor("ident", [128, 128], FP32, kind="ExternalInput")
    identb = nc.dram_tensor("identb", [128, 128], DT_MM, kind="ExternalInput")
    ones32 = nc.dram_tensor("ones32", [1, BC], DT_MM, kind="ExternalInput")
    dones = nc.dram_tensor("dones", [4, 128], DT_MM, kind="ExternalInput")
    donesu = nc.dram_tensor("donesu", [4, U * 128], DT_MM, kind="ExternalInput")
    h0t0 = nc.dram_tensor("h0t0", [128, 128], DT_MM, kind="ExternalInput")
    h1t0 = nc.dram_tensor("h1t0", [128, 128], DT_MM, kind="ExternalInput")
    woutt = nc.dram_tensor("woutt", [128, 4], DT_MM, kind="ExternalInput")
    bout = nc.dram_tensor("bout", [1, 1], DT_MM, kind="ExternalInput")
    y = nc.dram_tensor("y", [BC, 1], FP32, kind="ExternalOutput")

    with PatchedTileContext(nc) as tc:
        with (
            tc.tile_pool(name="perm", bufs=1) as perm,
            tc.tile_pool(name="work", bufs=3) as work,
            tc.tile_pool(name="gpsum", bufs=2, space="PSUM") as gpsum,
            tc.tile_pool(name="tpsum", bufs=2, space="PSUM") as tpsum,
        ):
            # ---- persistent tiles ----
            W1S = perm.tile([128, 4 * 3 * H], DT_MM, tag="W1S")
            W2I = perm.tile([128, 4 * 3 * H], DT_MM, tag="W2I")
            W2H = perm.tile([128, 4 * 3 * H], DT_MM, tag="W2H")
            F1 = perm.tile([8, H], DT_MM, tag="F1")
            F2 = perm.tile([4, H], DT_MM, tag="F2")
            ID = perm.tile([128, 128], FP32, tag="ID")
            IDB = perm.tile([128, 128], DT_MM, tag="IDB")
            ONES = perm.tile([1, BC], DT_MM, tag="ONES")
            XC = perm.tile([8, U * 128], DT_MM, tag="XC")
            DONES = perm.tile([4, 128], DT_MM, tag="DONES")
            WOUTT = perm.tile([128, 4], DT_MM, tag="WOUTT")
            BOUT = perm.tile([1, 1], DT_MM, tag="BOUT")
            h0t = [perm.tile([128, 128], DT_MM, name=f"h0t{i}", tag=f"h0t{i}") for i in range(2)]
            h1t = [perm.tile([128, 128], DT_MM, name=f"h1t{i}", tag=f"h1t{i}") for i in range(2)]

            for dst, src in [
                (W1S, w1s), (W2I, w2i), (W2H, w2h), (F1, f1), (F2, f2),
                (ID, ident), (IDB, identb), (ONES, ones32), (WOUTT, woutt), (BOUT, bout),
                (h0t[0], h0t0), (h1t[0], h1t0), (DONES, dones),
            ]:
                nc.gpsimd.dma_start(dst[:], src[:])
            nc.gpsimd.memset(XC[:], 0.0)
            # static ones-diagonal rows of the layer-1 x-carrier
            for j in range(4):
                nc.gpsimd.dma_start(XC[2 * j + 1:2 * j + 2, :],
                                    donesu[j:j + 1, :])

            w1v = W1S.rearrange("p (k g c) -> p k g c", k=4, g=3)
            w2iv = W2I.rearrange("p (k g c) -> p k g c", k=4, g=3)
            w2hv = W2H.rearrange("p (k g c) -> p k g c", k=4, g=3)

            def gate_mms(gp, hin_t, wv, f_t, f_lhsT, first, last, h0_side):
                """Emit col-tiled MMs for one layer's gates into psum tile gp.

                Strip free-layout blocks: [hn | r | z | in], 128 cols each.
                h-side MMs cover (hn, r, z) = cols 0:384; the layer-2 h0 side
                covers (r, z, in) = cols 128:512. Both are one N=384 MM per
                (j, k) so f32r streams at full rate (needs N >= 256).
                """
                if first:
                    # diagonal fold: one K<=8 matmul covers all four strips
                    nc.tensor.matmul(
                        gp[:, :], f_lhsT, f_t[:, :],
                        start=True, stop=False, tile_position=(0, 0),
                        skip_group_check=True,
                    )
                for k in range(4):
                    for j in range(4):
                        strip = gp[32 * j:32 * j + 32, :]
                        sview = strip.rearrange("p (g c) -> p g c", c=128)
                        tp = (0, 32 * j)
                        lhsT = hin_t[:, 32 * k:32 * k + 32]
                        cs = slice(128 * j, 128 * j + 128)
                        out = sview[:, 1:4, :] if h0_side else sview[:, 0:3, :]
                        nc.tensor.matmul(
                            out, lhsT, wv[:, k, 0:3, cs],
                            start=False, stop=(last and k == 3),
                            tile_position=tp, skip_group_check=True,
                        )

            def ew_head(gp, tag):
                """sig(z), sig(r), r*hn, +in  (ACT/DVE only, no PE)."""
                zs = work.tile([128, 128], BF16, tag=f"zs{tag}", name=f"zs{tag}")
                rs = work.tile([128, 128], BF16, tag=f"rs{tag}", name=f"rs{tag}")
                t1 = work.tile([128, 128], FP32, tag=f"t1{tag}", name=f"t1{tag}")
                npre = work.tile([128, 128], FP32, tag=f"np{tag}", name=f"np{tag}")
                nc.scalar.activation(zs[:], gp[:, 256:384], AF.Sigmoid)
                nc.scalar.activation(rs[:], gp[:, 128:256], AF.Sigmoid)
                nc.vector.tensor_mul(t1[:], rs[:], gp[:, 0:128])
                nc.vector.tensor_add(npre[:], t1[:], gp[:, 384:512])
                return {"zs": zs, "npre": npre}

            def ew_transpose(ew, tag):
                """PE transposes of n_pre and z (emitted when PE has slack).
                Both share one PSUM bank: z-T lives in a bf16 bitcast view."""
                tp = tpsum.tile([128, 192], FP32, tag=f"t{tag}",
                                name=f"t{tag}", bufs=1)
                tpn = tp[:, 0:128]
                tpz = tp[:, 128:192].bitcast(BF16)
                nc.tensor.transpose(tpn, ew["npre"][:], ID[:])
                nc.tensor.transpose(tpz, ew["zs"][:], IDB[:])
                ew["tpn"], ew["tpz"] = tpn, tpz

            def ew_tail(ew, hin_t, hout_t):
                """tanh -> h' = (1-z)*n + z*h, written to hout_t (bf16)."""
                nT = work.tile([128, 128], BF16, tag="nT", name="nT")
                zbT = work.tile([128, 128], BF16, tag="zbT", name="zbT")
                zhT = work.tile([128, 128], BF16, tag="zhT", name="zhT")
                nzb = work.tile([128, 128], BF16, tag="nzb", name="nzb")
                tpn, tpz = ew["tpn"], ew["tpz"]
                nc.scalar.activation(nT[:], tpn, AF.Tanh)
                # off-chain: zbT = 1 - zT ; zhT = zT * hT
                nc.vector.tensor_scalar(
                    zbT[:], tpz, -1.0, 1.0, ALU.mult, ALU.add
                )
                nc.vector.tensor_mul(zhT[:], tpz, hin_t[:])
                # chain tail
                nc.vector.tensor_mul(nzb[:], nT[:], zbT[:])
                nc.vector.tensor_add(hout_t[:], nzb[:], zhT[:])

            n_blocks = T // U
            with tc.For_i(0, repeat, name="rep") as _r:
              with tc.For_i(0, n_blocks) as i:
                  # stage this block's x^T rows into the diagonal x-carrier
                  # (nc.sync: SWDGE dma inside For_i fails this walrus build)
                  for j in range(4):
                      nc.sync.dma_start(
                          XC[2 * j:2 * j + 1, :].rearrange(
                              "p (u c) -> p u c",
                              c=128)[:, :, 32 * j:32 * j + 32],
                          xt[bass.ds(i * U, U), :],
                      )
                  # Software-pipelined emission: the PE stream per step is
                  #   g1(u) | T2(u-1) | g2h1(u) | T1(u) | g2h0(u)
                  # so layer-2's tail (tanh..h1') from step u-1 overlaps g1(u),
                  # and layer-1's tail overlaps g2's h1-side matmuls.
                  ew2_prev = None
                  for u in range(U):
                      pin, pout = u % 2, (u + 1) % 2
                      g1 = gpsum.tile([128, 512], FP32, tag="g1")
                      xl = XC[:, u * 128:(u + 1) * 128]
                      gate_mms(g1, h0t[pin], w1v, F1, xl,
                               first=True, last=True, h0_side=False)
                      if ew2_prev is not None:
                          ew_transpose(ew2_prev, "b")
                          ew_tail(ew2_prev, h1t[pout], h1t[pin])
                      g2 = gpsum.tile([128, 512], FP32, tag="g2")
                      gate_mms(g2, h1t[pin], w2hv, F2, DONES[:],
                               first=True, last=False, h0_side=False)
                      ew1 = ew_head(g1, "a")
                      ew_transpose(ew1, "a")
                      ew_tail(ew1, h0t[pin], h0t[pout])
                      gate_mms(g2, h0t[pout], w2iv, None, None,
                               first=False, last=True, h0_side=True)
                      ew2_prev = ew_head(g2, "b")
                  # drain the deferred layer-2 tail of the last step
                  ew_transpose(ew2_prev, "b")
                  ew_tail(ew2_prev, h1t[(U - 1) % 2], h1t[U % 2])

            # ---- final projection: y = h1 @ W_out.T + b_out ----
            po = tpsum.tile([32, 1], FP32, tag="po")
            nc.tensor.matmul(po[:], ONES[:], BOUT[:], start=True, stop=False,
                             skip_group_check=True)
            for k in range(4):
                nc.tensor.matmul(
                    po[:], h1t[0][:, 32 * k:32 * k + 32], WOUTT[:, k:k + 1],
                    start=False, stop=(k == 3), skip_group_check=True,
                )
            ysb = work.tile([32, 1], FP32, tag="ysb")
            nc.scalar.activation(ysb[:], po[:], AF.Copy)
            nc.gpsimd.dma_start(y[:], ysb[:])

    return nc


def _prep_core_inputs(xs, hidden0, hidden1, W_ih1, W_hh1, b_ih1, b_hh1,
                      W_ih2, W_hh2, b_ih2, b_hh2, W_out, b_out, U):
    """Host-side packing for one core's 32-row batch shard."""
    f = np.float32
    g = NP_MM
    T = xs.shape[1]

    def wT_pack(W, gorder):
        # [3H, H] -> [128, 4*3*512]: [p, k, g, c] = W[512*gorder[g]+c, 128k+p]
        Wg = W.reshape(3, H, 4, 128)[list(gorder)]
        return np.ascontiguousarray(
            Wg.transpose(3, 2, 0, 1).reshape(128, 4 * 3 * H)
        ).astype(g)

    def hT_pack(h):  # [32, 512] -> [128, 128] T-layout: [p, 32k+b] = h[b, 128k+p]
        return np.ascontiguousarray(
            h.reshape(BC, 4, 128).transpose(2, 1, 0).reshape(128, 128)
        ).astype(g)

    wi1 = W_ih1[:, 0]  # [1536]
    bsum1 = b_ih1 + b_hh1
    bsum2 = b_ih2 + b_hh2

    def blocks(vr, vz, vhn, vin):  # strip blocks in [hn | r | z | in] order
        out = np.zeros((4, 4, 128), f)
        for j in range(4):
            out[j, 0] = vhn[128 * j:128 * j + 128]
            out[j, 1] = vr[128 * j:128 * j + 128]
            out[j, 2] = vz[128 * j:128 * j + 128]
            out[j, 3] = vin[128 * j:128 * j + 128]
        return out.reshape(4 * H)

    xco = blocks(wi1[0:H], wi1[H:2 * H], np.zeros(H, f), wi1[2 * H:3 * H])
    bb1 = blocks(bsum1[0:H], bsum1[H:2 * H], b_hh1[2 * H:3 * H],
                 b_ih1[2 * H:3 * H])
    bb2 = blocks(bsum2[0:H], bsum2[H:2 * H], b_hh2[2 * H:3 * H],
                 b_ih2[2 * H:3 * H])
    # diagonal-fold carriers: F1 [8, 512] rows (2j = x-coefs, 2j+1 = biases)
    # for strip j; F2 [4, 512] row j = strip-j biases.
    f1 = np.zeros((8, H), f)
    f2 = np.zeros((4, H), f)
    for j in range(4):
        f1[2 * j] = xco[512 * j:512 * (j + 1)]
        f1[2 * j + 1] = bb1[512 * j:512 * (j + 1)]
        f2[j] = bb2[512 * j:512 * (j + 1)]
    dones = np.zeros((4, 128), f)
    for j in range(4):
        dones[j, 32 * j:32 * j + 32] = 1.0
    donesu = np.tile(dones, (1, U))

    return {
        "xt": np.ascontiguousarray(xs.T).astype(g),
        "w1s": wT_pack(W_hh1, (2, 0, 1)),
        "w2i": wT_pack(W_ih2, (0, 1, 2)),
        "w2h": wT_pack(W_hh2, (2, 0, 1)),
        "f1": f1.astype(g),
        "f2": f2.astype(g),
        "ident": np.eye(128, dtype=f),
        "identb": np.eye(128).astype(g),
        "ones32": np.ones((1, BC), g),
        "dones": dones.astype(g),
        "donesu": np.ascontiguousarray(donesu).astype(g),
        "h0t0": hT_pack(hidden0),
        "h1t0": hT_pack(hidden1),
        "woutt": np.ascontiguousarray(
            W_out[0].reshape(4, 128).T).astype(g),
        "bout": b_out.reshape(1, 1).astype(g),
    }


_LAST = {}


def kernel(x, hidden0, hidden1, W_ih1, W_hh1, b_ih1, b_hh1,
           W_ih2, W_hh2, b_ih2, b_hh2, W_out, b_out):
    x = np.asarray(x, np.float32)
    B, T = x.shape
    U = 16 if T % 16 == 0 else (8 if T % 8 == 0 else 4)
    args = [np.asarray(a, np.float32) for a in (
        W_ih1, W_hh1, b_ih1, b_hh1, W_ih2, W_hh2, b_ih2, b_hh2, W_out, b_out)]

    nc = build_nc(T, U)
    in_maps = []
    for c in range(N_CORES):
        sl = slice(c * BC, (c + 1) * BC)
        in_maps.append(_prep_core_inputs(
            x[sl], np.asarray(hidden0, np.float32)[sl],
            np.asarray(hidden1, np.float32)[sl], *args, U=U))

    res = run_bass_kernel_spmd(nc, in_maps, core_ids=list(range(N_CORES)))
    out = np.concatenate([res.results[c]["y"] for c in range(N_CORES)], axis=0)
    _LAST.update(nc=nc, in_maps=in_maps)

    if int(os.environ.get("GRU_BENCH", "0")):
        import time
        for rep in range(int(os.environ.get("GRU_BENCH", "0"))):
            t0 = time.time()
            run_bass_kernel_spmd(nc, in_maps, core_ids=list(range(N_CORES)))
            print(f"bench call {rep}: {(time.time()-t0)*1e3:.1f} ms")
    return out


def last_profile_ns():
    """Re-run the last-built kernel with NTFF tracing; return exec ns."""
    import tempfile
    tmpdir = tempfile.mkdtemp(prefix="gru_prof_")
    res = run_bass_kernel_spmd(
        _LAST["nc"], _LAST["in_maps"], core_ids=list(range(N_CORES)),
        trace=True, tmpdir=tmpdir,
    )
    print(f"profile tmpdir: {tmpdir}")
    if res.instructions_and_trace is not None:
        print(f"trace: {res.instructions_and_trace[1]}")
    print(f"mean exec: {res.mean_exec_time_ns} ns, "
          f"max core: {res.max_exec_time_core_id}")
    return res.exec_time_ns

